# revision 16
# baseline (speedup 1.0000x reference)
"""Trainium2 Bass kernel for GNN message passing (nn_Actor_26938034880699).

Strategy (self-contained, hardcoded shapes):
  - Host: partition edges by src-node range -> core k owns nodes
    [k*6250, (k+1)*6250) and all edges whose src falls there. The
    segment-sum then becomes core-local: NO collectives needed.
  - Within a core, edges are bucketed by 128-node tile of their src and
    padded to 128-edge chunks (a chunk never straddles a node tile).
  - Device per core (bf16 matmuls, f32 PSUM accumulate):
      L1:  hiddenT[h,e] = relu(W1ab.T @ xsdT + W1c.T @ eattrT + b1)
           (W1 stationary, edge features stream 512 wide)
      L2:  msg[e,h2] = hiddenT_chunk.T @ W2   (per 128-edge chunk)
      seg: aggT[h2,n] += msg_chunk.T @ onehot_chunk  (PSUM accum per node tile)
           onehot[e,n] = (src_rel[e] == n) generated on DVE via iota/is_equal
      heads: out[n,24] = softplus(xaugT_tile.T @ WHT + aggT_tile.T @ WHB)
           xaug = [x_feats; deg; ones] folds b2 (via deg) and head biases in.
  - Host: slice mu/sigma/alpha from the per-core [6272, 24] outputs.
"""

import os
import sys

sys.path.insert(0, "/opt/trn_rl_repo")

import numpy as np
import ml_dtypes

BF16 = ml_dtypes.bfloat16
FP8 = ml_dtypes.float8_e4m3

# Problem constants (hardcoded per task rules)
N = 50000
E = 800000
NODE = 64
EDGE = 32
HID = 128
OUT = 8
NCORES = 8
NPC = N // NCORES          # 6250 nodes per core
P = 128
NT = (NPC + P - 1) // P    # 49 node tiles per core
NTP = NT * P               # 6272 padded nodes per core

_BUILD_CACHE = {}
LAST_RESULT = None


def _install_ntff_hook():
    """This container's antenv lacks axon_hooks; synthesize it so
    run_bass_kernel_spmd(trace=True) can NTFF-profile via the axon .so."""
    import types
    try:
        import antenv.axon_hooks  # noqa: F401
        return
    except ImportError:
        pass
    try:
        import antenv
        from trn_agent_boot.trn_boot import _ntff_profile_via_ctypes
        mod = types.ModuleType("antenv.axon_hooks")
        _H = [None]
        mod.set_axon_ntff_profile_hook = lambda h: _H.__setitem__(0, h)
        mod.get_axon_ntff_profile_hook = lambda: _H[0]
        sys.modules["antenv.axon_hooks"] = mod
        antenv.axon_hooks = mod
        mod.set_axon_ntff_profile_hook(
            _ntff_profile_via_ctypes("/opt/axon/libaxon_pjrt.so"))
    except Exception:
        pass


_install_ntff_hook()


def _preprocess(x, src, dst, edge_attr):
    """Partition + pad edges. Returns per-core device arrays and chunk layout."""
    order = np.argsort(src, kind="stable")
    ssrc = src[order]
    sdst = dst[order]

    core = ssrc // NPC                       # [E] nondecreasing
    local = ssrc - core * NPC
    tilel = local // P                       # tile within core
    gt = core * NT + tilel                   # global (core,tile), nondecreasing

    cnt = np.bincount(gt, minlength=NCORES * NT).reshape(NCORES, NT)
    # per-tile chunk count, shared across cores (SPMD: one program)
    nch = np.maximum((cnt + P - 1) // P, 1).max(axis=0).astype(np.int64)  # [NT]
    # make total chunks a multiple of 8 (groups of 4, DMA per 2 groups)
    nchunk = int(nch.sum())
    nch[NT - 1] += (-nchunk) % 8
    nchunk = int(nch.sum())
    e_pad = nchunk * P

    choff = np.zeros(NT, np.int64)
    choff[1:] = np.cumsum(nch)[:-1]          # first chunk index of each tile

    # slot (padded position) of each sorted edge within its core's stream
    starts = np.searchsorted(gt, np.arange(NCORES * NT), side="left")
    rank = np.arange(E) - starts[gt]
    slot = choff[tilel] * P + rank           # [E]

    # gather tables with a zero row appended for padding
    xb = np.zeros((N + 1, NODE), BF16)
    xb[:N] = x.astype(BF16)
    eb = np.zeros((E + 1, EDGE), BF16)
    eb[:E] = edge_attr.astype(BF16)

    deg = np.bincount(src, minlength=N).astype(np.float32)

    XSD, EAT, SREL, XAUG = [], [], [], []
    for c in range(NCORES):
        m = core == c
        sl = slot[m]
        src_pad = np.full(e_pad, N, np.int64)
        dst_pad = np.full(e_pad, N, np.int64)
        att_pad = np.full(e_pad, E, np.int64)
        src_pad[sl] = ssrc[m]
        dst_pad[sl] = sdst[m]
        att_pad[sl] = order[m]

        xsd = np.empty((2 * NODE, e_pad), BF16)
        xsd[:NODE] = np.ascontiguousarray(xb[src_pad].T)
        xsd[NODE:] = np.ascontiguousarray(xb[dst_pad].T)
        XSD.append(xsd)
        EAT.append(np.ascontiguousarray(eb[att_pad].T))
        # host one-hot: [e_pad, P] scatter of ones -> [P(e), nchunk*P(n)]
        ohf = np.zeros((e_pad, P), BF16)
        ohf[sl, (local[m] % P)] = 1
        SREL.append(np.ascontiguousarray(
            ohf.reshape(nchunk, P, P).transpose(1, 0, 2).reshape(P, e_pad)))

        xa = np.zeros((NODE + 2, NTP), np.float32)
        lo = c * NPC
        xa[:NODE, :NPC] = x[lo:lo + NPC].T
        xa[NODE, :NPC] = deg[lo:lo + NPC]
        xa[NODE + 1, :] = 1.0
        XAUG.append(xa.astype(BF16))

    chunk_tile = np.repeat(np.arange(NT), nch)   # [nchunk]
    return XSD, EAT, SREL, XAUG, tuple(nch.tolist()), chunk_tile


def _build(nch):
    """Build (once per chunk layout) the SPMD Bass program."""
    if nch in _BUILD_CACHE:
        return _BUILD_CACHE[nch]

    import concourse.bass as bass
    import concourse.tile as tile
    import concourse.mybir as mybir
    from contextlib import ExitStack

    nchunk = int(sum(nch))
    e_pad = nchunk * P
    G = nchunk // 4                    # 512-edge groups
    chunk_tile = np.repeat(np.arange(NT), nch)
    tile_first = np.zeros(NT, np.int64)
    tile_first[1:] = np.cumsum(nch)[:-1]
    tile_last = np.cumsum(nch) - 1

    bf = mybir.dt.bfloat16
    f32 = mybir.dt.float32

    nc = bass.Bass("TRN2", target_bir_lowering=False, debug=False,
                   num_devices=NCORES)

    # constb columns: w1ab 0:128 | w2 128:256 | iota 256:384 | whb 384:408
    #                 wht 408:432 (66 rows) | w1c 432:560 (32 rows)
    xsd_d = nc.dram_tensor("xsd", [2 * NODE, e_pad], bf, kind="ExternalInput")
    eat_d = nc.dram_tensor("eat", [EDGE, e_pad], bf, kind="ExternalInput")
    oh_d = nc.dram_tensor("oh", [P, e_pad], bf, kind="ExternalInput")
    b1_d = nc.dram_tensor("b1", [P, 1], f32, kind="ExternalInput")
    xaug_d = nc.dram_tensor("xaug", [NODE + 2, NTP], bf, kind="ExternalInput")
    constb_d = nc.dram_tensor("constb", [P, 560], bf, kind="ExternalInput")
    out_d = nc.dram_tensor("out", [NTP, 3 * OUT], f32, kind="ExternalOutput")

    with tile.TileContext(nc) as tc, ExitStack() as ctx:
        const = ctx.enter_context(tc.tile_pool(name="const", bufs=1))
        xsd_p = ctx.enter_context(tc.tile_pool(name="xsd", bufs=6))
        eat_p = ctx.enter_context(tc.tile_pool(name="eat", bufs=6))
        hid_p = ctx.enter_context(tc.tile_pool(name="hid", bufs=4))
        msg_p = ctx.enter_context(tc.tile_pool(name="msg", bufs=4))
        msge_p = ctx.enter_context(tc.tile_pool(name="msge", bufs=4))
        oh_p = ctx.enter_context(tc.tile_pool(name="oh", bufs=6))
        asb_p = ctx.enter_context(tc.tile_pool(name="asb", bufs=2))
        hd_p = ctx.enter_context(tc.tile_pool(name="hd", bufs=8))
        psH_p = ctx.enter_context(tc.tile_pool(name="psH", bufs=2, space="PSUM"))
        psM_p = ctx.enter_context(tc.tile_pool(name="psM", bufs=3, space="PSUM"))
        psA_p = ctx.enter_context(tc.tile_pool(name="psA", bufs=2, space="PSUM"))
        psD_p = ctx.enter_context(tc.tile_pool(name="psD", bufs=1, space="PSUM"))

        def cload(dram, shape, dtype):
            t = const.tile(shape, dtype, tag=dram.name, name=dram.name + "_sb")
            nc.gpsimd.dma_start(t[:], dram.ap()[:, :])
            return t

        cb_t = cload(constb_d, [P, 560], bf)
        b1t_t = cload(b1_d, [P, 1], f32)
        xaug_t = cload(xaug_d, [NODE + 2, NTP], bf)
        w1ab_t = cb_t[:, 0:128]
        w2_t = cb_t[:, 128:256]
        iota_t = cb_t[:, 256:384]
        whb_t = cb_t[:, 384:408]
        wht_t = cb_t[0:NODE + 2, 408:432]
        w1c_t = cb_t[0:EDGE, 432:560]
        b1_t = b1t_t[:]

        agg_live = {}

        for gg in range(G // 2):
            xsd_t = xsd_p.tile([2 * NODE, 1024], bf, name="xsd_t")
            nc.sync.dma_start(xsd_t[:], xsd_d.ap()[:, bass.ts(gg, 1024)])
            eat_t = eat_p.tile([EDGE, 1024], bf, name="eat_t")
            nc.sync.dma_start(eat_t[:], eat_d.ap()[:, bass.ts(gg, 1024)])
            ohg_t = oh_p.tile([P, 1024], bf, name="ohg_t")
            nc.sync.dma_start(ohg_t[:], oh_d.ap()[:, bass.ts(gg, 1024)])

            for sub in range(2):
                g = 2 * gg + sub
                psH = psH_p.tile([HID, 512], f32, space="PSUM", name="psH")
                nc.tensor.matmul(out=psH[:], lhsT=w1ab_t,
                                 rhs=xsd_t[:, sub * 512:(sub + 1) * 512],
                                 start=True, stop=False, skip_group_check=True)
                nc.tensor.matmul(out=psH[:], lhsT=w1c_t,
                                 rhs=eat_t[:, sub * 512:(sub + 1) * 512],
                                 start=False, stop=True, skip_group_check=True)
                hid_t = hid_p.tile([HID, 512], bf, name="hid_t")
                nc.scalar.activation(hid_t[:], psH[:],
                                     bass.mybir.ActivationFunctionType.Relu,
                                     bias=b1_t)

                psM = psM_p.tile([P, 512], f32, space="PSUM", name="psM")
                nc.tensor.matmul(out=psM[:], lhsT=w2_t, rhs=hid_t[:],
                                 start=True, stop=True, skip_group_check=True)
                msgT_t = msg_p.tile([P, 512], bf, name="msgT_t")
                nc.vector.tensor_copy(msgT_t[:], psM[:])
                msg_t = msge_p.tile([P, 512], bf, name="msg_t")
                for k in range(4):
                    nc.sync.dma_start_transpose(
                        msg_t[:, k * P:(k + 1) * P],
                        msgT_t[:, k * P:(k + 1) * P])

                oh_t = ohg_t[:, sub * 512:(sub + 1) * 512]

                for k in range(4):
                    c = 4 * g + k
                    t = int(chunk_tile[c])
                    if c == tile_first[t]:
                        agg_live[t] = psA_p.tile([HID, P], f32, space="PSUM", name="agg")
                    nc.tensor.matmul(out=agg_live[t][:],
                                     lhsT=msg_t[:, k * P:(k + 1) * P],
                                     rhs=oh_t[:, k * P:(k + 1) * P],
                                     start=(c == tile_first[t]),
                                     stop=(c == tile_last[t]),
                                     skip_group_check=True)
                    if c == tile_last[t]:
                        # finalize node tile t: heads + output
                        agg_sb = asb_p.tile([HID, P], bf, name="agg_sb")
                        nc.vector.tensor_copy(agg_sb[:], agg_live[t][:])
                        del agg_live[t]
                        psD = psD_p.tile([P, 3 * OUT], f32, space="PSUM", name="psD")
                        nc.tensor.matmul(out=psD[:],
                                         lhsT=xaug_t[:, t * P:(t + 1) * P],
                                         rhs=wht_t, start=True, stop=False,
                                         skip_group_check=True)
                        nc.tensor.matmul(out=psD[:], lhsT=agg_sb[:],
                                         rhs=whb_t, start=False, stop=True,
                                         skip_group_check=True)
                        # softplus(v) = ln(exp(v) + 1); Softplus has no ACT
                        # table on this compiler, ln/exp/relu share one set
                        ex_t = hd_p.tile([P, 3 * OUT], f32, name="ex_t")
                        nc.scalar.activation(
                            ex_t[:], psD[:],
                            bass.mybir.ActivationFunctionType.Exp)
                        hd_t = hd_p.tile([P, 3 * OUT], f32, name="hd_t")
                        nc.scalar.activation(
                            hd_t[:], ex_t[:],
                            bass.mybir.ActivationFunctionType.Ln, bias=1.0)
                        nc.gpsimd.dma_start(
                            out_d.ap()[t * P:(t + 1) * P, :], hd_t[:])

    # walrus's per-struct embedded-wait capacity is tiny (1 for ACT/TS ops,
    # 2 for DMA). Hoist excess waits into single-wait NOPs on the same
    # engine right before the instruction (program order makes this safe).
    keep = (mybir.InstNoOp, mybir.InstUnconditionalBranch,
            mybir.InstEventSemaphore, mybir.InstCall)
    f = nc.m.functions[0]
    for blk in f.blocks:
        newlist = []
        for inst in blk.instructions:
            si = inst.sync_info
            if (si is not None and si.on_wait and len(si.on_wait) > 1
                    and not isinstance(inst, keep)):
                for w in si.on_wait[:-1]:
                    nop = mybir.InstNoOp(
                        name=nc.get_next_instruction_name(),
                        ins=[], outs=[],
                        sync_info=mybir.SyncInfo(on_wait=[w], on_update=[]),
                        bass_nofuse=True,
                        engine=inst.engine)
                    newlist.append(nop)
                inst.sync_info = mybir.SyncInfo(
                    on_wait=[si.on_wait[-1]], on_update=si.on_update)
            newlist.append(inst)
        blk.instructions[:] = newlist

    _BUILD_CACHE[nch] = nc
    return nc


def kernel(**inputs):
    global LAST_RESULT
    x = np.asarray(inputs["x"], np.float32)
    edge_index = np.asarray(inputs["edge_index"])
    edge_attr = np.asarray(inputs["edge_attr"], np.float32)
    W1 = np.asarray(inputs["W1"], np.float32)
    b1 = np.asarray(inputs["b1"], np.float32)
    W2 = np.asarray(inputs["W2"], np.float32)
    b2 = np.asarray(inputs["b2"], np.float32)
    Wmu = np.asarray(inputs["Wmu"], np.float32)
    bmu = np.asarray(inputs["bmu"], np.float32)
    Wsig = np.asarray(inputs["Wsig"], np.float32)
    bsig = np.asarray(inputs["bsig"], np.float32)
    Wc = np.asarray(inputs["Wc"], np.float32)
    bc = np.asarray(inputs["bc"], np.float32)
    nf = int(np.asarray(inputs["num_factories"]))

    src = edge_index[0].astype(np.int64)
    dst = edge_index[1].astype(np.int64)

    XSD, EAT, SREL, XAUG, nch, _ = _preprocess(x, src, dst, edge_attr)

    # fold b2 and head biases: head = x@Wh_top + agg_raw@Wh_bot + deg*(b2@Wh_bot) + bh
    wht = np.zeros((NODE + 2, 3 * OUT), np.float32)
    whb = np.zeros((HID, 3 * OUT), np.float32)
    for i, (Wh, bh) in enumerate([(Wmu, bmu), (Wsig, bsig), (Wc, bc)]):
        wht[:NODE, i * OUT:(i + 1) * OUT] = Wh[:NODE]
        wht[NODE, i * OUT:(i + 1) * OUT] = b2 @ Wh[NODE:]
        wht[NODE + 1, i * OUT:(i + 1) * OUT] = bh
        whb[:, i * OUT:(i + 1) * OUT] = Wh[NODE:]

    constb = np.zeros((P, 560), np.float32)
    constb[:, 0:128] = W1[:2 * NODE]
    constb[:, 128:256] = W2
    constb[:, 256:384] = np.broadcast_to(np.arange(P, dtype=np.float32), (P, P))
    constb[:HID, 384:408] = whb
    constb[:NODE + 2, 408:432] = wht
    constb[:EDGE, 432:560] = W1[2 * NODE:]
    constb = constb.astype(BF16)

    b1c = np.ascontiguousarray(b1.reshape(P, 1)).astype(np.float32)
    in_maps = []
    for c in range(NCORES):
        in_maps.append({"xsd": XSD[c], "eat": EAT[c], "oh": SREL[c],
                        "b1": b1c, "xaug": XAUG[c], "constb": constb})

    nc = _build(nch)

    from concourse.bass_utils import run_bass_kernel_spmd
    res = run_bass_kernel_spmd(nc, in_maps, core_ids=list(range(NCORES)))
    LAST_RESULT = res

    Os = [r["out"] for r in res.results]
    mu = np.ascontiguousarray(Os[0][:nf, 0:OUT])
    sigma = np.ascontiguousarray(Os[0][:nf, OUT:2 * OUT])
    alpha = np.concatenate(
        [Os[0][nf:NPC, 2 * OUT:]] + [Os[c][:NPC, 2 * OUT:] for c in range(1, NCORES)],
        axis=0)
    return (mu, sigma, alpha)


# revision 18
# speedup vs baseline: 7.0910x; 7.0910x over previous
"""Trainium2 Bass kernel for GNN message passing (nn_Actor_26938034880699).

Strategy (self-contained, hardcoded shapes):
  - Host: partition edges by src-node range -> core k owns nodes
    [k*6250, (k+1)*6250) and all edges whose src falls there. The
    segment-sum then becomes core-local: NO collectives needed.
  - Within a core, edges are bucketed by 128-node tile of their src and
    padded to 128-edge chunks (a chunk never straddles a node tile).
  - Device per core (bf16 matmuls, f32 PSUM accumulate):
      L1:  hiddenT[h,e] = relu(W1ab.T @ xsdT + W1c.T @ eattrT + b1)
           (W1 stationary, edge features stream 512 wide)
      L2:  msg[e,h2] = hiddenT_chunk.T @ W2   (per 128-edge chunk)
      seg: aggT[h2,n] += msg_chunk.T @ onehot_chunk  (PSUM accum per node tile)
           onehot[e,n] = (src_rel[e] == n) generated on DVE via iota/is_equal
      heads: out[n,24] = softplus(xaugT_tile.T @ WHT + aggT_tile.T @ WHB)
           xaug = [x_feats; deg; ones] folds b2 (via deg) and head biases in.
  - Host: slice mu/sigma/alpha from the per-core [6272, 24] outputs.
"""

import os
import sys

sys.path.insert(0, "/opt/trn_rl_repo")

import numpy as np
import ml_dtypes

BF16 = ml_dtypes.bfloat16
FP8 = ml_dtypes.float8_e4m3

# Problem constants (hardcoded per task rules)
N = 50000
E = 800000
NODE = 64
EDGE = 32
HID = 128
OUT = 8
NCORES = 8
NPC = N // NCORES          # 6250 nodes per core
P = 128
NT = (NPC + P - 1) // P    # 49 node tiles per core
NTP = NT * P               # 6272 padded nodes per core

_BUILD_CACHE = {}
LAST_RESULT = None


def _install_ntff_hook():
    """This container's antenv lacks axon_hooks; synthesize it so
    run_bass_kernel_spmd(trace=True) can NTFF-profile via the axon .so."""
    import types
    try:
        import antenv.axon_hooks  # noqa: F401
        return
    except ImportError:
        pass
    try:
        import antenv
        from trn_agent_boot.trn_boot import _ntff_profile_via_ctypes
        mod = types.ModuleType("antenv.axon_hooks")
        _H = [None]
        mod.set_axon_ntff_profile_hook = lambda h: _H.__setitem__(0, h)
        mod.get_axon_ntff_profile_hook = lambda: _H[0]
        sys.modules["antenv.axon_hooks"] = mod
        antenv.axon_hooks = mod
        mod.set_axon_ntff_profile_hook(
            _ntff_profile_via_ctypes("/opt/axon/libaxon_pjrt.so"))
    except Exception:
        pass


_install_ntff_hook()


def _preprocess(x, src, dst, edge_attr):
    """Partition + pad edges. Returns per-core device arrays and chunk layout."""
    order = np.argsort(src, kind="stable")
    ssrc = src[order]
    sdst = dst[order]

    core = ssrc // NPC                       # [E] nondecreasing
    local = ssrc - core * NPC
    tilel = local // P                       # tile within core
    gt = core * NT + tilel                   # global (core,tile), nondecreasing

    cnt = np.bincount(gt, minlength=NCORES * NT).reshape(NCORES, NT)
    # per-tile chunk count, shared across cores (SPMD: one program)
    nch = np.maximum((cnt + P - 1) // P, 1).max(axis=0).astype(np.int64)  # [NT]
    # make total chunks a multiple of 8 (groups of 4, DMA per 2 groups)
    nchunk = int(nch.sum())
    nch[NT - 1] += (-nchunk) % 16
    nchunk = int(nch.sum())
    e_pad = nchunk * P

    choff = np.zeros(NT, np.int64)
    choff[1:] = np.cumsum(nch)[:-1]          # first chunk index of each tile

    # slot (padded position) of each sorted edge within its core's stream
    starts = np.searchsorted(gt, np.arange(NCORES * NT), side="left")
    rank = np.arange(E) - starts[gt]
    slot = choff[tilel] * P + rank           # [E]

    # gather tables with a zero row appended for padding
    xb = np.zeros((N + 1, NODE), BF16)
    xb[:N] = x.astype(BF16)
    eb = np.zeros((E + 1, EDGE), BF16)
    eb[:E] = edge_attr.astype(BF16)

    deg = np.bincount(src, minlength=N).astype(np.float32)

    XSD, EAT, SREL, XAUG = [], [], [], []
    for c in range(NCORES):
        m = core == c
        sl = slot[m]
        src_pad = np.full(e_pad, N, np.int64)
        dst_pad = np.full(e_pad, N, np.int64)
        att_pad = np.full(e_pad, E, np.int64)
        src_pad[sl] = ssrc[m]
        dst_pad[sl] = sdst[m]
        att_pad[sl] = order[m]

        xsd = np.empty((2 * NODE, e_pad), BF16)
        xsd[:NODE] = np.ascontiguousarray(xb[src_pad].T)
        xsd[NODE:] = np.ascontiguousarray(xb[dst_pad].T)
        XSD.append(xsd)
        EAT.append(np.ascontiguousarray(eb[att_pad].T))
        # host one-hot: [e_pad, P] scatter of ones -> [P(e), nchunk*P(n)]
        ohf = np.zeros((e_pad, P), BF16)
        ohf[sl, (local[m] % P)] = 1
        SREL.append(np.ascontiguousarray(
            ohf.reshape(nchunk, P, P).transpose(1, 0, 2).reshape(P, e_pad)))

        xa = np.zeros((NODE + 2, NTP), np.float32)
        lo = c * NPC
        xa[:NODE, :NPC] = x[lo:lo + NPC].T
        xa[NODE, :NPC] = deg[lo:lo + NPC]
        xa[NODE + 1, :] = 1.0
        XAUG.append(xa.astype(BF16))

    chunk_tile = np.repeat(np.arange(NT), nch)   # [nchunk]
    return XSD, EAT, SREL, XAUG, tuple(nch.tolist()), chunk_tile


def _build(nch):
    """Build (once per chunk layout) the SPMD Bass program."""
    if nch in _BUILD_CACHE:
        return _BUILD_CACHE[nch]

    import concourse.bass as bass
    import concourse.tile as tile
    import concourse.mybir as mybir
    from contextlib import ExitStack

    nchunk = int(sum(nch))
    e_pad = nchunk * P
    G = nchunk // 4                    # 512-edge groups
    chunk_tile = np.repeat(np.arange(NT), nch)
    tile_first = np.zeros(NT, np.int64)
    tile_first[1:] = np.cumsum(nch)[:-1]
    tile_last = np.cumsum(nch) - 1

    bf = mybir.dt.bfloat16
    f32 = mybir.dt.float32

    nc = bass.Bass("TRN2", target_bir_lowering=False, debug=False,
                   num_devices=NCORES)

    # constb columns: w1ab 0:128 | w2 128:256 | iota 256:384 | whb 384:408
    #                 wht 408:432 (66 rows) | w1c 432:560 (32 rows)
    xsd_d = nc.dram_tensor("xsd", [2 * NODE, e_pad], bf, kind="ExternalInput")
    eat_d = nc.dram_tensor("eat", [EDGE, e_pad], bf, kind="ExternalInput")
    oh_d = nc.dram_tensor("oh", [P, e_pad], bf, kind="ExternalInput")
    b1_d = nc.dram_tensor("b1", [P, 1], f32, kind="ExternalInput")
    xaug_d = nc.dram_tensor("xaug", [NODE + 2, NTP], bf, kind="ExternalInput")
    constb_d = nc.dram_tensor("constb", [P, 560], bf, kind="ExternalInput")
    out_d = nc.dram_tensor("out", [NTP, 3 * OUT], f32, kind="ExternalOutput")

    with tile.TileContext(nc) as tc, ExitStack() as ctx:
        const = ctx.enter_context(tc.tile_pool(name="const", bufs=1))
        xsd_p = ctx.enter_context(tc.tile_pool(name="xsd", bufs=3))
        eat_p = ctx.enter_context(tc.tile_pool(name="eat", bufs=3))
        hid_p = ctx.enter_context(tc.tile_pool(name="hid", bufs=4))
        msg_p = ctx.enter_context(tc.tile_pool(name="msg", bufs=4))
        oh_p = ctx.enter_context(tc.tile_pool(name="oh", bufs=3))
        asb_p = ctx.enter_context(tc.tile_pool(name="asb", bufs=2))
        hd_p = ctx.enter_context(tc.tile_pool(name="hd", bufs=8))
        psH_p = ctx.enter_context(tc.tile_pool(name="psH", bufs=2, space="PSUM"))
        psM_p = ctx.enter_context(tc.tile_pool(name="psM", bufs=3, space="PSUM"))
        psA_p = ctx.enter_context(tc.tile_pool(name="psA", bufs=2, space="PSUM"))
        psD_p = ctx.enter_context(tc.tile_pool(name="psD", bufs=1, space="PSUM"))

        def cload(dram, shape, dtype):
            t = const.tile(shape, dtype, tag=dram.name, name=dram.name + "_sb")
            nc.gpsimd.dma_start(t[:], dram.ap()[:, :])
            return t

        cb_t = cload(constb_d, [P, 560], bf)
        b1t_t = cload(b1_d, [P, 1], f32)
        xaug_t = cload(xaug_d, [NODE + 2, NTP], bf)
        w1ab_t = cb_t[:, 0:128]
        w2_t = cb_t[:, 128:256]
        iota_t = cb_t[:, 256:384]
        whb_t = cb_t[:, 384:408]
        wht_t = cb_t[0:NODE + 2, 408:432]
        w1c_t = cb_t[0:EDGE, 432:560]
        b1_t = b1t_t[:]

        agg_live = {}

        for gg4 in range(G // 4):
            xsd_t = xsd_p.tile([2 * NODE, 2048], bf, name="xsd_t")
            nc.sync.dma_start(xsd_t[:], xsd_d.ap()[:, bass.ts(gg4, 2048)])
            eat_t = eat_p.tile([EDGE, 2048], bf, name="eat_t")
            nc.sync.dma_start(eat_t[:], eat_d.ap()[:, bass.ts(gg4, 2048)])
            ohg_t = oh_p.tile([P, 2048], bf, name="ohg_t")
            nc.scalar.dma_start(ohg_t[:], oh_d.ap()[:, bass.ts(gg4, 2048)])

            for sub in range(4):
                g = 4 * gg4 + sub
                psH = psH_p.tile([HID, 512], f32, space="PSUM", name="psH")
                nc.tensor.matmul(out=psH[:], lhsT=w1ab_t,
                                 rhs=xsd_t[:, sub * 512:(sub + 1) * 512],
                                 start=True, stop=False, skip_group_check=True)
                nc.tensor.matmul(out=psH[:], lhsT=w1c_t,
                                 rhs=eat_t[:, sub * 512:(sub + 1) * 512],
                                 start=False, stop=True, skip_group_check=True)
                hid_t = hid_p.tile([HID, 512], bf, name="hid_t")
                nc.scalar.activation(hid_t[:], psH[:],
                                     bass.mybir.ActivationFunctionType.Relu,
                                     bias=b1_t)

                psM = psM_p.tile([P, 512], f32, space="PSUM", name="psM")
                for k in range(4):
                    nc.tensor.matmul(out=psM[:, k * P:(k + 1) * P],
                                     lhsT=hid_t[:, k * P:(k + 1) * P],
                                     rhs=w2_t, start=True, stop=True,
                                     skip_group_check=True)
                msg_t = msg_p.tile([P, 512], bf, name="msg_t")
                nc.vector.tensor_copy(msg_t[:], psM[:])

                oh_t = ohg_t[:, sub * 512:(sub + 1) * 512]

                for k in range(4):
                    c = 4 * g + k
                    t = int(chunk_tile[c])
                    if c == tile_first[t]:
                        agg_live[t] = psA_p.tile([HID, P], f32, space="PSUM", name="agg")
                    nc.tensor.matmul(out=agg_live[t][:],
                                     lhsT=msg_t[:, k * P:(k + 1) * P],
                                     rhs=oh_t[:, k * P:(k + 1) * P],
                                     start=(c == tile_first[t]),
                                     stop=(c == tile_last[t]),
                                     skip_group_check=True)
                    if c == tile_last[t]:
                        # finalize node tile t: heads + output
                        agg_sb = asb_p.tile([HID, P], bf, name="agg_sb")
                        nc.vector.tensor_copy(agg_sb[:], agg_live[t][:])
                        del agg_live[t]
                        psD = psD_p.tile([P, 3 * OUT], f32, space="PSUM", name="psD")
                        nc.tensor.matmul(out=psD[:],
                                         lhsT=xaug_t[:, t * P:(t + 1) * P],
                                         rhs=wht_t, start=True, stop=False,
                                         skip_group_check=True)
                        nc.tensor.matmul(out=psD[:], lhsT=agg_sb[:],
                                         rhs=whb_t, start=False, stop=True,
                                         skip_group_check=True)
                        # softplus(v) = ln(exp(v) + 1); Softplus has no ACT
                        # table on this compiler, ln/exp/relu share one set
                        ex_t = hd_p.tile([P, 3 * OUT], f32, name="ex_t")
                        nc.scalar.activation(
                            ex_t[:], psD[:],
                            bass.mybir.ActivationFunctionType.Exp)
                        hd_t = hd_p.tile([P, 3 * OUT], f32, name="hd_t")
                        nc.scalar.activation(
                            hd_t[:], ex_t[:],
                            bass.mybir.ActivationFunctionType.Ln, bias=1.0)
                        nc.gpsimd.dma_start(
                            out_d.ap()[t * P:(t + 1) * P, :], hd_t[:])

    # walrus's per-struct embedded-wait capacity is tiny (1 for ACT/TS ops,
    # 2 for DMA). Hoist excess waits into single-wait NOPs on the same
    # engine right before the instruction (program order makes this safe).
    keep = (mybir.InstNoOp, mybir.InstUnconditionalBranch,
            mybir.InstEventSemaphore, mybir.InstCall)
    f = nc.m.functions[0]
    for blk in f.blocks:
        newlist = []
        for inst in blk.instructions:
            si = inst.sync_info
            if (si is not None and si.on_wait and len(si.on_wait) > 1
                    and not isinstance(inst, keep)):
                for w in si.on_wait[:-1]:
                    nop = mybir.InstNoOp(
                        name=nc.get_next_instruction_name(),
                        ins=[], outs=[],
                        sync_info=mybir.SyncInfo(on_wait=[w], on_update=[]),
                        bass_nofuse=True,
                        engine=inst.engine)
                    newlist.append(nop)
                inst.sync_info = mybir.SyncInfo(
                    on_wait=[si.on_wait[-1]], on_update=si.on_update)
            newlist.append(inst)
        blk.instructions[:] = newlist

    _BUILD_CACHE[nch] = nc
    return nc


def kernel(**inputs):
    global LAST_RESULT
    x = np.asarray(inputs["x"], np.float32)
    edge_index = np.asarray(inputs["edge_index"])
    edge_attr = np.asarray(inputs["edge_attr"], np.float32)
    W1 = np.asarray(inputs["W1"], np.float32)
    b1 = np.asarray(inputs["b1"], np.float32)
    W2 = np.asarray(inputs["W2"], np.float32)
    b2 = np.asarray(inputs["b2"], np.float32)
    Wmu = np.asarray(inputs["Wmu"], np.float32)
    bmu = np.asarray(inputs["bmu"], np.float32)
    Wsig = np.asarray(inputs["Wsig"], np.float32)
    bsig = np.asarray(inputs["bsig"], np.float32)
    Wc = np.asarray(inputs["Wc"], np.float32)
    bc = np.asarray(inputs["bc"], np.float32)
    nf = int(np.asarray(inputs["num_factories"]))

    src = edge_index[0].astype(np.int64)
    dst = edge_index[1].astype(np.int64)

    XSD, EAT, SREL, XAUG, nch, _ = _preprocess(x, src, dst, edge_attr)

    # fold b2 and head biases: head = x@Wh_top + agg_raw@Wh_bot + deg*(b2@Wh_bot) + bh
    wht = np.zeros((NODE + 2, 3 * OUT), np.float32)
    whb = np.zeros((HID, 3 * OUT), np.float32)
    for i, (Wh, bh) in enumerate([(Wmu, bmu), (Wsig, bsig), (Wc, bc)]):
        wht[:NODE, i * OUT:(i + 1) * OUT] = Wh[:NODE]
        wht[NODE, i * OUT:(i + 1) * OUT] = b2 @ Wh[NODE:]
        wht[NODE + 1, i * OUT:(i + 1) * OUT] = bh
        whb[:, i * OUT:(i + 1) * OUT] = Wh[NODE:]

    constb = np.zeros((P, 560), np.float32)
    constb[:, 0:128] = W1[:2 * NODE]
    constb[:, 128:256] = W2
    constb[:, 256:384] = np.broadcast_to(np.arange(P, dtype=np.float32), (P, P))
    constb[:HID, 384:408] = whb
    constb[:NODE + 2, 408:432] = wht
    constb[:EDGE, 432:560] = W1[2 * NODE:]
    constb = constb.astype(BF16)

    b1c = np.ascontiguousarray(b1.reshape(P, 1)).astype(np.float32)
    in_maps = []
    for c in range(NCORES):
        in_maps.append({"xsd": XSD[c], "eat": EAT[c], "oh": SREL[c],
                        "b1": b1c, "xaug": XAUG[c], "constb": constb})

    nc = _build(nch)

    from concourse.bass_utils import run_bass_kernel_spmd
    res = run_bass_kernel_spmd(nc, in_maps, core_ids=list(range(NCORES)))
    LAST_RESULT = res

    Os = [r["out"] for r in res.results]
    mu = np.ascontiguousarray(Os[0][:nf, 0:OUT])
    sigma = np.ascontiguousarray(Os[0][:nf, OUT:2 * OUT])
    alpha = np.concatenate(
        [Os[0][nf:NPC, 2 * OUT:]] + [Os[c][:NPC, 2 * OUT:] for c in range(1, NCORES)],
        axis=0)
    return (mu, sigma, alpha)
